# revision 35
# baseline (speedup 1.0000x reference)
"""Self-contained Trainium2 Bass kernel: causal multi-head attention
(B=2, N=2048, E=1024, H=16, D=64) distributed over 8 NeuronCores.

Entry point: kernel(**inputs) -> np.ndarray of shape (2, 2048, 1024).

Sharding: tensor-parallel over heads (2 heads per core, x replicated).
Each core runs QKV projection, causal attention (scores kept transposed,
softmax denominators via a ones-column appended to V), and a partial
output projection over its 128 columns of the H*D dimension; the host
sums the 8 partials and adds the output bias.

The matmul datapath is bf16 end-to-end (same 1 cycle/row as fp32r but
half the PE power — sustained fp32r trips the chip's power throttler to
K=4/8 — plus half the DMA traffic and SBUF footprint).
"""

# ---------------------------------------------------------------------------
# workaround 1: walrus in this container accepts at most ~1-2 semaphore waits
# per instruction; split Tile's final drain per-proc and hoist excess waits
# onto same-engine NoOps.
# ---------------------------------------------------------------------------
import re
from concourse.vector_clock import ScopedClock, VectorClock
import concourse.tile as tile


def _split_drain_and_barrier(self, tick_clock, wait_clock):
    g = tick_clock.global_clock
    ticks = [int(t) for t in re.findall(r"-?\d+", repr(g))]
    for proc, t in enumerate(ticks):
        if t <= 0:
            continue
        v = VectorClock()
        v.require_at_least(proc, t)
        nop = self.nc.sync.nop(nofuse=True, hint="drain_split_wait")
        wait_clock.add_sem_waits(nop.ins, ScopedClock({None: v}))
    self.nc.sync.drain()
    self.nc.all_engine_barrier(sem_only=True)
    assert self.sems is not None
    popped = self.nc._tile_sem_poison_stack.pop()
    assert popped is self._sem_poison
    # NOTE: the stock path clears every Tile semaphore one EVENT_SEMAPHORE at
    # a time and re-barriers (~7us of teardown inside the measured window).
    # The NEFF executes exactly once per launch, so skip the clear.


def _apply_tile_drain_patch():
    tile.TileContext._drain_and_barrier = _split_drain_and_barrier


import concourse.mybir as mybir

# conservative per-opcode wait capacity
_CAPS = {
    "EventSemaphore": 2,
}
_DEFAULT_CAP = 1

_counter = [0]


def _split_excess_waits(nc):
    for f in nc.m.functions:
        for bb in f.blocks:
            insts = bb.instructions  # live list
            i = 0
            while i < len(insts):
                inst = insts[i]
                si = inst.sync_info
                if si is None:
                    i += 1
                    continue
                waits = list(si.on_wait)
                cap = _CAPS.get(inst.opcode, _DEFAULT_CAP)
                if len(waits) <= cap:
                    i += 1
                    continue
                excess, keep = waits[:-cap], waits[-cap:]
                for w in excess:
                    _counter[0] += 1
                    nop = mybir.InstNoOp(name=f"WSPLIT-{_counter[0]}")
                    nop.engine = inst.engine
                    nop.sync_info = mybir.SyncInfo(on_wait=[w], on_update=[])
                    insts.insert(i, nop)
                    i += 1
                inst.sync_info = mybir.SyncInfo(on_wait=keep, on_update=list(si.on_update))
                i += 1
    return nc


# ---------------------------------------------------------------------------
# the kernel itself
# ---------------------------------------------------------------------------
from contextlib import ExitStack

import numpy as np

import concourse.bass as bass
import concourse.mybir as mybir
import concourse.tile as tile
from concourse.masks import make_identity

f32 = mybir.dt.float32
bf16 = mybir.dt.bfloat16
AF = mybir.ActivationFunctionType

B, N, E = 2, 2048, 1024
H, D = 16, 64
NCORES = 8
HPC = H // NCORES          # heads per core = 2
D2 = HPC * D               # 128 packed head dims per core
SCALE = D ** -0.5
NB = N // 512              # 4 query/key 512-blocks per b
EC = E // 128              # 8 contraction chunks
JC = N // 128              # 16 key 128-chunks per b
VW = D + 1                 # V width with the ones column

# tuning knobs (overridden by sweep harness)
CONFIG = {
    "psum": "222",       # per-pool PSUM bufs (scores pairs get 2x2 banks)
    "lag": 8,            # PV queue depth in chunks (Exp-latency hiding)
    "inter": 1,          # interleave qkv(b1) into attn(b0) as micro-op filler
    "op1fill": 1,        # feed outproj(b1) into late attn(b1) per query block
    "recip": "matmul",   # broadcast 1/denom via PE outer-product (no DMA)
    "otraw": 1,          # copy o_ps to SBUF early, freeing the PSUM bank
    "prefetch": 1,       # reorder qkv loads two groups ahead
    "pair": 1,           # merge exp of two full chunks into one [128,1024] op
    "defer_nb3": 0,      # move qkv(b1) nb3 + last V-transposes into attn(b1)
    "ycopy": "vector",   # engine for outproj psum->sbuf copies
    "pool_norm": 1,      # normalize/cmask muls on gpsimd (Pool) engine
}


def _merge(primary, filler, ratio):
    """Round-robin merge: ~`ratio` primary units per filler unit."""
    out = []
    fi = 0
    acc = 0.0
    for i, u in enumerate(primary):
        out.append(u)
        acc += 1.0
        while acc >= ratio and fi < len(filler):
            out.append(filler[fi])
            fi += 1
            acc -= ratio
    out.extend(filler[fi:])
    return out


def build_nc() -> bass.Bass:
    nc = bass.Bass()
    # xb: host-repacked x so block (b, nb) is one contiguous [128, EC*512]
    # slab -> a single clean DMA per block instead of 8 strided ones.
    xb = nc.declare_dram_parameter("xb", [B * NB * 128, EC * 512], bf16, isOutput=False)
    # wqkv repacked to [128, EC*384] matching the wt SBUF layout.
    wqkv = nc.declare_dram_parameter("wqkv", [128, EC * 3 * D2], bf16, isOutput=False)
    bias2 = nc.declare_dram_parameter("bias2", [D2, 3], f32, isOutput=False)
    wo = nc.declare_dram_parameter("wo", [D2, E], bf16, isOutput=False)
    cmask = nc.declare_dram_parameter("cmask", [128, 128], bf16, isOutput=False)
    y = nc.declare_dram_parameter("y", [B * N, E], bf16, isOutput=True)

    with tile.TileContext(nc) as tc, ExitStack() as ctx:
        const = ctx.enter_context(tc.tile_pool(name="const", bufs=1))
        xt_pool = ctx.enter_context(tc.tile_pool(name="xt", bufs=4))
        big = ctx.enter_context(tc.tile_pool(name="big", bufs=2))
        pt_pool = ctx.enter_context(tc.tile_pool(name="pt", bufs=6))
        r_pool = ctx.enter_context(tc.tile_pool(name="r", bufs=4))
        y_pool = ctx.enter_context(tc.tile_pool(name="y", bufs=3))
        dram = ctx.enter_context(tc.tile_pool(name="dram", bufs=3, space="DRAM"))
        if CONFIG["psum"] == "shared8":
            psum = ctx.enter_context(tc.tile_pool(name="psum", bufs=8, space="PSUM"))
            psum2 = psum
            psumo = psum
        else:
            b1_, b2_, b3_ = (int(c) for c in CONFIG["psum"])
            psum = ctx.enter_context(tc.tile_pool(name="psum", bufs=b1_, space="PSUM"))
            psum2 = ctx.enter_context(tc.tile_pool(name="psum2", bufs=b2_, space="PSUM"))
            psumo = ctx.enter_context(tc.tile_pool(name="psumo", bufs=b3_, space="PSUM"))

        # constants: wt + bias first (they gate the first QKV matmuls);
        # cm/wo are only needed later and issue after the second x load.
        wt = const.tile([128, EC * 3 * D2], bf16, tag="wt")  # per-chunk [128, 384]
        nc.sync.dma_start(wt[:], wqkv[:])
        b2_t = const.tile([D2, 3], f32, tag="b2")
        nc.sync.dma_start(b2_t[:], bias2[:])
        wo_t = const.tile([D2, E], bf16, tag="wo")
        cm_t = const.tile([128, 128], bf16, tag="cm")

        def late_consts():
            nc.sync.dma_start(cm_t[:], cmask[:])
            nc.sync.dma_start(wo_t[:], wo[:])

        ident = const.tile([128, 128], bf16, tag="id")
        make_identity(nc, ident[:])
        ones_r = const.tile([1, D], bf16, tag="ones_r")
        nc.gpsimd.memset(ones_r[:], 1.0)

        # per-b persistent tiles
        QT = [big.tile([D2, N], bf16, tag="qt", name=f"QT{b}") for b in range(B)]
        KT = [big.tile([D2, N], bf16, tag="kt", name=f"KT{b}") for b in range(B)]
        VT = [big.tile([D2, N], bf16, tag="vt", name=f"VT{b}") for b in range(B)]
        vaug = [big.tile([128, HPC * JC * VW], bf16, tag="vaug", name=f"va{b}") for b in range(B)]
        OT = [big.tile([D2, N], bf16, tag="ot", name=f"OT{b}") for b in range(B)]

        def qkv_microops(b):
            """One closure ~= one PE instruction (or a DMA/copy side-op)."""
            ops = []
            for nb in range(NB):
                xcol = xt_pool.tile([128, EC * 512], bf16, tag="xt", name=f"xc{b}_{nb}")
                slab = (b * NB + nb) * 128

                def load(b=b, nb=nb, xcol=xcol, slab=slab):
                    if b == 0 and nb == 0:
                        # ramp: two half-slices let the first matmuls start
                        # early without serializing 8 issue slots on SP
                        half = EC * 256
                        nc.sync.dma_start(xcol[:, 0:half], xb[slab : slab + 128, 0:half])
                        nc.sync.dma_start(xcol[:, half:], xb[slab : slab + 128, half:])
                    else:
                        nc.sync.dma_start(xcol[:], xb[slab : slab + 128, :])
                    if b == 0 and nb == 1:
                        late_consts()

                load._nb = nb
                ops.append(load)
                for pi in range(3):
                    cell = {}

                    for e in range(EC):
                        def mm(pi=pi, xcol=xcol, cell=cell, e=e):
                            if e == 0:
                                cell["ps"] = psum.tile([128, 512], f32, tag="ps", name=f"qps{id(cell)}")
                            nc.tensor.matmul(
                                cell["ps"][:],
                                lhsT=wt[:, e * 384 + pi * 128 : e * 384 + (pi + 1) * 128],
                                rhs=xcol[:, e * 512 : (e + 1) * 512],
                                start=(e == 0),
                                stop=(e == EC - 1),
                            )

                        mm._nb = nb
                        ops.append(mm)

                    def cp(b=b, nb=nb, pi=pi, cell=cell):
                        dst = (QT[b], KT[b], VT[b])[pi]
                        nc.vector.tensor_add(
                            dst[:, nb * 512 : (nb + 1) * 512],
                            cell["ps"][:],
                            b2_t[:, pi : pi + 1].to_broadcast((128, 512)),
                        )

                    cp._nb = nb
                    ops.append(cp)

            def vinit(b=b):
                nc.gpsimd.memset(vaug[b][:], 1.0)

            ops.append(vinit)
            for jc in range(JC):
                def vtrans(b=b, jc=jc):
                    tp = psum.tile([128, 128], bf16, tag="ps", name=f"tp{b}_{jc}")
                    nc.tensor.transpose(
                        tp[:], VT[b][:, jc * 128 : (jc + 1) * 128], ident[:]
                    )
                    for h in range(HPC):
                        nc.vector.tensor_copy(
                            vaug[b][:, (h * JC + jc) * VW : (h * JC + jc) * VW + D],
                            tp[:, h * D : (h + 1) * D],
                        )

                vtrans._nb = 3 if jc >= 12 else jc // 4
                ops.append(vtrans)
            if not CONFIG["prefetch"]:
                return ops
            # prefetch: pull each nb's load two groups ahead of its matmuls
            loads = [op for op in ops if op.__name__ == "load"]
            rest = [op for op in ops if op.__name__ != "load"]
            out = loads[:2]
            li = 2
            per_group = 27  # 24 mms + 3 copies
            for i, op in enumerate(rest):
                out.append(op)
                if li < len(loads) and i == per_group * (li - 1):
                    out.append(loads[li])
                    li += 1
            return out

        def outproj_microops(b):
            ops = []
            for nt in range(N // 128):
                cell = {}
                for ec in range(E // 512):
                    def op(b=b, nt=nt, ec=ec, cell=cell):
                        if ec == 0:
                            cell["ys"] = y_pool.tile([128, E], bf16, tag="y", name=f"ys{b}_{nt}")
                        ys = cell["ys"]
                        yp = psum.tile([128, 512], f32, tag="ps", name=f"yp{b}_{nt}_{ec}")
                        nc.tensor.matmul(
                            yp[:],
                            lhsT=OT[b][:, nt * 128 : (nt + 1) * 128],
                            rhs=wo_t[:, ec * 512 : (ec + 1) * 512],
                            start=True,
                            stop=True,
                        )
                        # psum->sbuf cast; alternate engines for b1 so the
                        # tail drains two copies in parallel
                        if CONFIG["ycopy"] == "split" and b == 1 and ec == 1:
                            nc.scalar.copy(ys[:, ec * 512 : (ec + 1) * 512], yp[:])
                        else:
                            nc.vector.tensor_copy(ys[:, ec * 512 : (ec + 1) * 512], yp[:])
                        if ec == E // 512 - 1:
                            nc.sync.dma_start(
                                y[b * N + nt * 128 : b * N + (nt + 1) * 128, :],
                                ys[:],
                            )

                    ops.append(op)
            return ops

        pending_fin = []

        def emit_attention(b, filler, rate, post_block=None):
            """Emit attention for batch b. The two heads' K=64 score matmuls
            are issued back-to-back as 64x128 row tiles (T0 = SBUF partitions
            0-63, T8 = 64-127) so they run CONCURRENTLY in the two halves of
            the PE array; one Exp covers both heads. PV matmuls fire from a
            queue `lag` entries behind so the Exp latency hides behind later
            score matmuls; `rate` filler micro-ops per chunk keep the PE
            dense so the HAM clock-gate stays open."""
            fi = [0.0]
            avg_njc = NCHUNKS / (N // 512)

            def fill(njc=None):
                # uniform draw per chunk: each chunk's PE work (640ns) trails
                # its Exp (1038ns) by the same margin, so dense blocks need
                # fillers just as much as sparse ones (per-block equalizing
                # starved the njc=16 blocks and tripped the HAM clock-gate)
                fi[0] += rate
                while fi[0] >= 1.0 and filler:
                    filler.pop(0)()
                    fi[0] -= 1.0

            mul_eng = nc.gpsimd if CONFIG["pool_norm"] else nc.vector

            # densest block first: keeps the PE stream dense at the
            # qkv->attention seam, and leaves the sparse q0=0 block for the
            # end where outproj fillers are plentiful (short tail)
            for q0 in reversed(range(0, N, 512)):
                njc = q0 // 128 + 4
                o_ps = [
                    psumo.tile([128, 512], f32, tag="ps" if psumo is psum else "ops", name=f"o{b}_{h}_{q0}")
                    for h in range(HPC)
                ]
                pvq = []

                def fire(keep, pvq=pvq):
                    while len(pvq) > keep:
                        pvq.pop(0)()

                for jc in range(njc):
                    rel = jc * 128 - q0
                    c0 = max(rel, 0)
                    w = 512 - c0
                    s_ps = psum2.tile([128, 1024], f32, tag="ps" if psum2 is psum else "ps2", name=f"s{b}_{q0}_{jc}")
                    pt = pt_pool.tile([128, 1024], bf16, tag="pt", name=f"p{b}_{q0}_{jc}")
                    for h in range(HPC):
                        nc.tensor.matmul(
                            s_ps[:, h * 512 : h * 512 + w],
                            lhsT=KT[b][h * D : (h + 1) * D, jc * 128 : (jc + 1) * 128],
                            rhs=QT[b][h * D : (h + 1) * D, q0 + c0 : q0 + 512],
                            start=True,
                            stop=True,
                            tile_position=(h * D, 0),
                        )
                    if jc == 0:
                        while pending_fin:
                            pending_fin.pop(0)()
                    if w == 512:
                        nc.scalar.activation(pt[:], s_ps[:], AF.Exp, scale=SCALE)
                    else:
                        for h in range(HPC):
                            nc.scalar.activation(
                                pt[:, h * 512 : h * 512 + w],
                                s_ps[:, h * 512 : h * 512 + w],
                                AF.Exp,
                                scale=SCALE,
                            )
                    if rel >= 0:
                        for h in range(HPC):
                            mul_eng.tensor_mul(
                                pt[:, h * 512 : h * 512 + 128],
                                pt[:, h * 512 : h * 512 + 128],
                                cm_t[:],
                            )
                    fill(njc)
                    fire(CONFIG["lag"])

                    for h in range(HPC):
                        def pv(h=h, q0=q0, jc=jc, njc=njc, o_ps=o_ps[h], pt=pt, c0=c0, w=w, b=b):
                            nc.tensor.matmul(
                                o_ps[0:VW, c0:512],
                                lhsT=vaug[b][:, (h * JC + jc) * VW : (h * JC + jc + 1) * VW],
                                rhs=pt[:, h * 512 : h * 512 + w],
                                start=(jc == 0),
                                stop=(jc == njc - 1),
                            )

                        pvq.append(pv)
                fire(0)
                fill()

                for h in range(HPC):
                    hp = slice(h * D, (h + 1) * D)
                    ops = o_ps[h]
                    # 1/denom = exp(-ln(denom)) on ACT: ln and exp share the
                    # natural_log_exp_and_others table (no table-swap cost);
                    # DVE reciprocal on a [1,512] AP would be 3.3us.
                    t1 = r_pool.tile([1, 512], f32, tag="t1", name=f"t1{b}_{h}_{q0}")
                    nc.scalar.activation(t1[:], ops[D : D + 1, :], AF.Ln)
                    r1 = r_pool.tile([1, 512], bf16, tag="r1", name=f"r1{b}_{h}_{q0}")
                    nc.scalar.activation(r1[:], t1[:], AF.Exp, scale=-1.0)
                    ot_raw = pt_pool.tile([D, 512], f32, tag="ot_raw", name=f"or{b}_{h}_{q0}")
                    nc.vector.tensor_copy(ot_raw[:], ops[0:D, :])

                    def fin(b=b, hp=hp, q0=q0, o_ps=ops, r1=r1, ot_raw=ot_raw):
                        # rank-1 PE broadcast of 1/denom into the spare rows
                        # of the same PSUM bank, then the divide. Deferred to
                        # the next block's start so the PE queue never
                        # head-of-line blocks on the reciprocal.
                        nc.tensor.matmul(
                            o_ps[D : D + D, :],
                            lhsT=ones_r[:],
                            rhs=r1[:],
                            start=True,
                            stop=True,
                        )
                        nc.vector.tensor_mul(
                            OT[b][hp, q0 : q0 + 512], ot_raw[:], o_ps[D : D + D, :]
                        )

                    pending_fin.append(fin)
                if post_block is not None:
                    filler.extend(post_block(q0))
            # drain leftovers: fins FIRST — leftover fillers include outproj
            # ops that read the OT slices the deferred fins write
            while pending_fin:
                pending_fin.pop(0)()
            while filler:
                filler.pop(0)()

        NCHUNKS = sum(q0 // 128 + 4 for q0 in range(0, N, 512))  # 40 per b

        if not CONFIG["inter"]:
            for b in range(B):
                for u in qkv_microops(b):
                    u()
                emit_attention(b, [], rate=0.0)
                for u in outproj_microops(b):
                    u()
        else:
            for u in qkv_microops(0):
                u()
            f2 = qkv_microops(1)
            tail1 = []
            if CONFIG["defer_nb3"]:
                tail1 = [u for u in f2 if getattr(u, "_nb", 0) == 3]
                f2 = [u for u in f2 if getattr(u, "_nb", 0) != 3]
            emit_attention(0, f2, rate=len(f2) / NCHUNKS)
            f3 = tail1 + outproj_microops(0)
            op1 = outproj_microops(1)
            if CONFIG["op1fill"]:
                def release_op1(q0):
                    lo = q0 // 512 * 8
                    return op1[lo : lo + 8]
                emit_attention(
                    1, f3, rate=(len(f3) + len(op1)) / NCHUNKS, post_block=release_op1
                )
            else:
                emit_attention(1, f3, rate=len(f3) / NCHUNKS)
                for u in op1:
                    u()
    return nc


NP_BF16 = mybir.dt.np(bf16)


def prep_in_maps(x, Wqkv, bqkv, Wout, bout):
    x = np.ascontiguousarray(np.asarray(x, dtype=np.float32))
    Wqkv = np.asarray(Wqkv, dtype=np.float32)
    bqkv = np.asarray(bqkv, dtype=np.float32)
    Wout = np.asarray(Wout, dtype=np.float32)

    # xb: block (b, nb) as a contiguous [128, EC*512] slab; xb[slab+p, e*512+t]
    # = x[b, nb*512+t, e*128+p]
    xb = np.ascontiguousarray(
        x.reshape(B, NB, 512, EC, 128).transpose(0, 1, 4, 3, 2).reshape(B * NB * 128, EC * 512)
    ).astype(NP_BF16)
    Wr = Wqkv.reshape(H, D, 3, E)
    br = bqkv.reshape(H, D, 3)

    j = np.arange(128)[:, None]
    q = np.arange(128)[None, :]
    cmask = (j <= q).astype(NP_BF16)

    in_maps = []
    for c in range(NCORES):
        hs = [HPC * c + i for i in range(HPC)]
        wqkv_c = np.concatenate(
            [
                np.concatenate([Wr[h, :, p, :].T for h in hs], axis=1)
                for p in range(3)
            ],
            axis=1,
        )  # [E, 384]
        bias2_c = np.stack(
            [np.concatenate([br[h, :, p] for h in hs]) for p in range(3)], axis=1
        )  # [128, 3]
        wo_c = np.ascontiguousarray(Wout[:, hs[0] * D : (hs[-1] + 1) * D].T)  # [128, E]
        # wqkv_c is [E, 384] = [(e p), c]; repack to [p, (e c)] to match wt
        wqkv_p = wqkv_c.reshape(EC, 128, 3 * D2).transpose(1, 0, 2).reshape(128, EC * 3 * D2)
        in_maps.append(
            {
                "xb": xb,
                "wqkv": np.ascontiguousarray(wqkv_p).astype(NP_BF16),
                "bias2": np.ascontiguousarray(bias2_c),
                "wo": wo_c.astype(NP_BF16),
                "cmask": cmask,
            }
        )
    return in_maps


def assemble_output(results, bout):
    bout = np.asarray(bout, dtype=np.float32)
    y = np.zeros((B * N, E), dtype=np.float32)
    for res in results:
        y += np.asarray(res["y"], dtype=np.float32)
    y += bout[None, :]
    return y.reshape(B, N, E)


# ---------------------------------------------------------------------------
# NTFF profile hook shim (used only when _TRACE is set by a test harness)
# ---------------------------------------------------------------------------
import contextlib
import ctypes
import sys
import types

_SO_PATH = "/opt/axon/libaxon_pjrt.so"
_hook = None


def _make_hook():
    lib = ctypes.CDLL(_SO_PATH)
    if not hasattr(lib, "axon_start_nrt_profile"):
        return None
    lib.axon_start_nrt_profile.argtypes = [
        ctypes.POINTER(ctypes.c_int64),
        ctypes.c_size_t,
    ]
    lib.axon_start_nrt_profile.restype = ctypes.c_int64
    lib.axon_stop_nrt_profile.argtypes = [ctypes.c_char_p]
    lib.axon_stop_nrt_profile.restype = ctypes.c_int64

    @contextlib.contextmanager
    def _profile(output_dir, device_ids):
        import jax

        jax.devices()
        if device_ids:
            ids = (ctypes.c_int64 * len(device_ids))(*device_ids)
            rc = lib.axon_start_nrt_profile(ids, len(device_ids))
        else:
            rc = lib.axon_start_nrt_profile(None, 0)
        if rc != 0:
            raise RuntimeError(f"axon_start_nrt_profile rc={rc}")
        try:
            yield
        finally:
            n = lib.axon_stop_nrt_profile(str(output_dir).encode())
            if n < 0:
                raise RuntimeError(f"axon_stop_nrt_profile rc={n}")
            print(f"profile: {n} file(s) written to {output_dir}")

    return _profile


def _install_ntff_hook():
    global _hook
    _hook = _make_hook()
    mod = types.ModuleType("antenv.axon_hooks")
    mod.get_axon_ntff_profile_hook = lambda: _hook

    def set_axon_ntff_profile_hook(h):
        global _hook
        _hook = h

    mod.set_axon_ntff_profile_hook = set_axon_ntff_profile_hook
    sys.modules["antenv.axon_hooks"] = mod


# ---------------------------------------------------------------------------
# host-side entry point
# ---------------------------------------------------------------------------
_NC = None
_TRACE = False
_TRACE_DIR = "/tmp/attn_kernel_trace"
LAST_EXEC_NS = None


def _get_nc():
    global _NC
    if _NC is None:
        _apply_tile_drain_patch()
        nc = build_nc()
        _split_excess_waits(nc)
        _NC = nc
    return _NC


def kernel(x, Wqkv, bqkv, Wout, bout):
    global LAST_EXEC_NS
    from concourse.bass_utils import run_bass_kernel_spmd

    nc = _get_nc()
    in_maps = prep_in_maps(x, Wqkv, bqkv, Wout, bout)
    kwargs = {}
    if _TRACE:
        import shutil

        shutil.rmtree(_TRACE_DIR, ignore_errors=True)
        _install_ntff_hook()
        kwargs = {"trace": True, "tmpdir": _TRACE_DIR}
    res = run_bass_kernel_spmd(nc, in_maps, list(range(NCORES)), **kwargs)
    LAST_EXEC_NS = res.exec_time_ns
    return assemble_output(res.results, bout)


# revision 37
# speedup vs baseline: 1.0614x; 1.0614x over previous
"""Self-contained Trainium2 Bass kernel: causal multi-head attention
(B=2, N=2048, E=1024, H=16, D=64) distributed over 8 NeuronCores.

Entry point: kernel(**inputs) -> np.ndarray of shape (2, 2048, 1024).

Sharding: tensor-parallel over heads (2 heads per core, x replicated).
Each core runs QKV projection, causal attention (scores kept transposed,
softmax denominators via a ones-column appended to V), and a partial
output projection over its 128 columns of the H*D dimension; the host
sums the 8 partials and adds the output bias.

The matmul datapath is bf16 end-to-end (same 1 cycle/row as fp32r but
half the PE power — sustained fp32r trips the chip's power throttler to
K=4/8 — plus half the DMA traffic and SBUF footprint).
"""

# ---------------------------------------------------------------------------
# workaround 1: walrus in this container accepts at most ~1-2 semaphore waits
# per instruction; split Tile's final drain per-proc and hoist excess waits
# onto same-engine NoOps.
# ---------------------------------------------------------------------------
import re
from concourse.vector_clock import ScopedClock, VectorClock
import concourse.tile as tile


def _split_drain_and_barrier(self, tick_clock, wait_clock):
    g = tick_clock.global_clock
    ticks = [int(t) for t in re.findall(r"-?\d+", repr(g))]
    for proc, t in enumerate(ticks):
        if t <= 0:
            continue
        v = VectorClock()
        v.require_at_least(proc, t)
        nop = self.nc.sync.nop(nofuse=True, hint="drain_split_wait")
        wait_clock.add_sem_waits(nop.ins, ScopedClock({None: v}))
    self.nc.sync.drain()
    self.nc.all_engine_barrier(sem_only=True)
    assert self.sems is not None
    popped = self.nc._tile_sem_poison_stack.pop()
    assert popped is self._sem_poison
    # NOTE: the stock path clears every Tile semaphore one EVENT_SEMAPHORE at
    # a time and re-barriers (~7us of teardown inside the measured window).
    # The NEFF executes exactly once per launch, so skip the clear.


def _apply_tile_drain_patch():
    tile.TileContext._drain_and_barrier = _split_drain_and_barrier


import concourse.mybir as mybir

# conservative per-opcode wait capacity
_CAPS = {
    "EventSemaphore": 2,
}
_DEFAULT_CAP = 1

_counter = [0]


def _split_excess_waits(nc):
    for f in nc.m.functions:
        for bb in f.blocks:
            insts = bb.instructions  # live list
            i = 0
            while i < len(insts):
                inst = insts[i]
                si = inst.sync_info
                if si is None:
                    i += 1
                    continue
                waits = list(si.on_wait)
                cap = _CAPS.get(inst.opcode, _DEFAULT_CAP)
                if len(waits) <= cap:
                    i += 1
                    continue
                excess, keep = waits[:-cap], waits[-cap:]
                for w in excess:
                    _counter[0] += 1
                    nop = mybir.InstNoOp(name=f"WSPLIT-{_counter[0]}")
                    nop.engine = inst.engine
                    nop.sync_info = mybir.SyncInfo(on_wait=[w], on_update=[])
                    insts.insert(i, nop)
                    i += 1
                inst.sync_info = mybir.SyncInfo(on_wait=keep, on_update=list(si.on_update))
                i += 1
    return nc


# ---------------------------------------------------------------------------
# the kernel itself
# ---------------------------------------------------------------------------
from contextlib import ExitStack

import numpy as np

import concourse.bass as bass
import concourse.mybir as mybir
import concourse.tile as tile
from concourse.masks import make_identity

f32 = mybir.dt.float32
bf16 = mybir.dt.bfloat16
AF = mybir.ActivationFunctionType

B, N, E = 2, 2048, 1024
H, D = 16, 64
NCORES = 8
HPC = H // NCORES          # heads per core = 2
D2 = HPC * D               # 128 packed head dims per core
SCALE = D ** -0.5
NB = N // 512              # 4 query/key 512-blocks per b
EC = E // 128              # 8 contraction chunks
JC = N // 128              # 16 key 128-chunks per b
VW = D + 1                 # V width with the ones column

# tuning knobs (overridden by sweep harness)
CONFIG = {
    "psum": "222",       # per-pool PSUM bufs (scores pairs get 2x2 banks)
    "lag": 8,            # PV queue depth in chunks (Exp-latency hiding)
    "inter": 1,          # interleave qkv(b1) into attn(b0) as micro-op filler
    "op1fill": 1,        # feed outproj(b1) into late attn(b1) per query block
    "recip": "matmul",   # broadcast 1/denom via PE outer-product (no DMA)
    "otraw": 1,          # copy o_ps to SBUF early, freeing the PSUM bank
    "prefetch": 1,       # reorder qkv loads two groups ahead
    "pair": 1,           # merge exp of two full chunks into one [128,1024] op
    "defer_nb3": 0,      # move qkv(b1) nb3 + last V-transposes into attn(b1)
    "ycopy": "vector",   # engine for outproj psum->sbuf copies
    "pool_norm": 1,      # normalize/cmask muls on gpsimd (Pool) engine
}


def _merge(primary, filler, ratio):
    """Round-robin merge: ~`ratio` primary units per filler unit."""
    out = []
    fi = 0
    acc = 0.0
    for i, u in enumerate(primary):
        out.append(u)
        acc += 1.0
        while acc >= ratio and fi < len(filler):
            out.append(filler[fi])
            fi += 1
            acc -= ratio
    out.extend(filler[fi:])
    return out


def build_nc() -> bass.Bass:
    nc = bass.Bass()
    # xb: host-repacked x so block (b, nb) is one contiguous [128, EC*512]
    # slab -> a single clean DMA per block instead of 8 strided ones.
    xb = nc.declare_dram_parameter("xb", [B * NB * 128, EC * 512], bf16, isOutput=False)
    # wqkv repacked to [128, EC*384] matching the wt SBUF layout.
    wqkv = nc.declare_dram_parameter("wqkv", [128, EC * 3 * D2], bf16, isOutput=False)
    bias2 = nc.declare_dram_parameter("bias2", [D2, 3], f32, isOutput=False)
    wo = nc.declare_dram_parameter("wo", [D2, E], bf16, isOutput=False)
    cmask = nc.declare_dram_parameter("cmask", [128, 128], bf16, isOutput=False)
    y = nc.declare_dram_parameter("y", [B * N, E], bf16, isOutput=True)

    with tile.TileContext(nc) as tc, ExitStack() as ctx:
        const = ctx.enter_context(tc.tile_pool(name="const", bufs=1))
        xt_pool = ctx.enter_context(tc.tile_pool(name="xt", bufs=4))
        big = ctx.enter_context(tc.tile_pool(name="big", bufs=2))
        pt_pool = ctx.enter_context(tc.tile_pool(name="pt", bufs=8))
        r_pool = ctx.enter_context(tc.tile_pool(name="r", bufs=4))
        y_pool = ctx.enter_context(tc.tile_pool(name="y", bufs=3))
        dram = ctx.enter_context(tc.tile_pool(name="dram", bufs=3, space="DRAM"))
        if CONFIG["psum"] == "shared8":
            psum = ctx.enter_context(tc.tile_pool(name="psum", bufs=8, space="PSUM"))
            psum2 = psum
            psumo = psum
        else:
            b1_, b2_, b3_ = (int(c) for c in CONFIG["psum"])
            psum = ctx.enter_context(tc.tile_pool(name="psum", bufs=b1_, space="PSUM"))
            psum2 = ctx.enter_context(tc.tile_pool(name="psum2", bufs=b2_, space="PSUM"))
            psumo = ctx.enter_context(tc.tile_pool(name="psumo", bufs=b3_, space="PSUM"))

        # constants: wt + bias first (they gate the first QKV matmuls);
        # cm/wo are only needed later and issue after the second x load.
        wt = const.tile([128, EC * 3 * D2], bf16, tag="wt")  # per-chunk [128, 384]
        nc.sync.dma_start(wt[:], wqkv[:])
        b2_t = const.tile([D2, 3], f32, tag="b2")
        nc.sync.dma_start(b2_t[:], bias2[:])
        wo_t = const.tile([D2, E], bf16, tag="wo")
        cm_t = const.tile([128, 128], bf16, tag="cm")

        def late_consts():
            nc.sync.dma_start(cm_t[:], cmask[:])
            nc.sync.dma_start(wo_t[:], wo[:])

        ident = const.tile([128, 128], bf16, tag="id")
        make_identity(nc, ident[:])
        ones_r = const.tile([1, D], bf16, tag="ones_r")
        nc.gpsimd.memset(ones_r[:], 1.0)

        # per-b persistent tiles
        QT = [big.tile([D2, N], bf16, tag="qt", name=f"QT{b}") for b in range(B)]
        KT = [big.tile([D2, N], bf16, tag="kt", name=f"KT{b}") for b in range(B)]
        VT = [big.tile([D2, N], bf16, tag="vt", name=f"VT{b}") for b in range(B)]
        vaug = [big.tile([128, HPC * JC * VW], bf16, tag="vaug", name=f"va{b}") for b in range(B)]
        OT = [big.tile([D2, N], bf16, tag="ot", name=f"OT{b}") for b in range(B)]

        def qkv_microops(b):
            """One closure ~= one PE instruction (or a DMA/copy side-op)."""
            ops = []
            for nb in range(NB):
                xcol = xt_pool.tile([128, EC * 512], bf16, tag="xt", name=f"xc{b}_{nb}")
                slab = (b * NB + nb) * 128

                def load(b=b, nb=nb, xcol=xcol, slab=slab):
                    if b == 0 and nb == 0:
                        # ramp: two half-slices let the first matmuls start
                        # early without serializing 8 issue slots on SP
                        half = EC * 256
                        nc.sync.dma_start(xcol[:, 0:half], xb[slab : slab + 128, 0:half])
                        nc.sync.dma_start(xcol[:, half:], xb[slab : slab + 128, half:])
                    else:
                        nc.sync.dma_start(xcol[:], xb[slab : slab + 128, :])
                    if b == 0 and nb == 1:
                        late_consts()

                load._nb = nb
                ops.append(load)
                for pi in range(3):
                    cell = {}

                    for e in range(EC):
                        def mm(pi=pi, xcol=xcol, cell=cell, e=e):
                            if e == 0:
                                cell["ps"] = psum.tile([128, 512], f32, tag="ps", name=f"qps{id(cell)}")
                            nc.tensor.matmul(
                                cell["ps"][:],
                                lhsT=wt[:, e * 384 + pi * 128 : e * 384 + (pi + 1) * 128],
                                rhs=xcol[:, e * 512 : (e + 1) * 512],
                                start=(e == 0),
                                stop=(e == EC - 1),
                            )

                        mm._nb = nb
                        ops.append(mm)

                    def cp(b=b, nb=nb, pi=pi, cell=cell):
                        dst = (QT[b], KT[b], VT[b])[pi]
                        nc.vector.tensor_add(
                            dst[:, nb * 512 : (nb + 1) * 512],
                            cell["ps"][:],
                            b2_t[:, pi : pi + 1].to_broadcast((128, 512)),
                        )

                    cp._nb = nb
                    ops.append(cp)

            def vinit(b=b):
                nc.gpsimd.memset(vaug[b][:], 1.0)

            ops.append(vinit)
            for jc in range(JC):
                def vtrans(b=b, jc=jc):
                    tp = psum.tile([128, 128], bf16, tag="ps", name=f"tp{b}_{jc}")
                    nc.tensor.transpose(
                        tp[:], VT[b][:, jc * 128 : (jc + 1) * 128], ident[:]
                    )
                    for h in range(HPC):
                        nc.vector.tensor_copy(
                            vaug[b][:, (h * JC + jc) * VW : (h * JC + jc) * VW + D],
                            tp[:, h * D : (h + 1) * D],
                        )

                vtrans._nb = 3 if jc >= 12 else jc // 4
                ops.append(vtrans)
            if not CONFIG["prefetch"]:
                return ops
            # prefetch: pull each nb's load two groups ahead of its matmuls
            loads = [op for op in ops if op.__name__ == "load"]
            rest = [op for op in ops if op.__name__ != "load"]
            out = loads[:2]
            li = 2
            per_group = 27  # 24 mms + 3 copies
            for i, op in enumerate(rest):
                out.append(op)
                if li < len(loads) and i == per_group * (li - 1):
                    out.append(loads[li])
                    li += 1
            return out

        def outproj_microops(b):
            ops = []
            for nt in range(N // 128):
                cell = {}
                for ec in range(E // 512):
                    def op(b=b, nt=nt, ec=ec, cell=cell):
                        if ec == 0:
                            cell["ys"] = y_pool.tile([128, E], bf16, tag="y", name=f"ys{b}_{nt}")
                        ys = cell["ys"]
                        yp = psum.tile([128, 512], f32, tag="ps", name=f"yp{b}_{nt}_{ec}")
                        nc.tensor.matmul(
                            yp[:],
                            lhsT=OT[b][:, nt * 128 : (nt + 1) * 128],
                            rhs=wo_t[:, ec * 512 : (ec + 1) * 512],
                            start=True,
                            stop=True,
                        )
                        # psum->sbuf cast. For the tail block (b1 rows 0-511,
                        # drained last under reversed q0 order) alternate
                        # DVE/ACT — ACT is idle there and the serial cast
                        # chain paces the final outproj drain.
                        if b == 1 and nt < 4 and ec == 1:
                            nc.scalar.copy(ys[:, ec * 512 : (ec + 1) * 512], yp[:])
                        else:
                            nc.vector.tensor_copy(ys[:, ec * 512 : (ec + 1) * 512], yp[:])
                        if ec == E // 512 - 1:
                            nc.sync.dma_start(
                                y[b * N + nt * 128 : b * N + (nt + 1) * 128, :],
                                ys[:],
                            )

                    ops.append(op)
            return ops

        pending_fin = []

        def emit_attention(b, filler, rate, post_block=None):
            """Emit attention for batch b. The two heads' K=64 score matmuls
            are issued back-to-back as 64x128 row tiles (T0 = SBUF partitions
            0-63, T8 = 64-127) so they run CONCURRENTLY in the two halves of
            the PE array; one Exp covers both heads. PV matmuls fire from a
            queue `lag` entries behind so the Exp latency hides behind later
            score matmuls; `rate` filler micro-ops per chunk keep the PE
            dense so the HAM clock-gate stays open."""
            fi = [0.0]
            avg_njc = NCHUNKS / (N // 512)

            def fill(njc=None):
                # draw proportionally more fillers in short blocks so the PE
                # stays dense enough to hold the HAM clock-gate open
                fi[0] += rate * (avg_njc / njc if njc else 1.0)
                while fi[0] >= 1.0 and filler:
                    filler.pop(0)()
                    fi[0] -= 1.0

            mul_eng = nc.gpsimd if CONFIG["pool_norm"] else nc.vector

            # densest block first: keeps the PE stream dense at the
            # qkv->attention seam, and leaves the sparse q0=0 block for the
            # end where outproj fillers are plentiful (short tail)
            for q0 in reversed(range(0, N, 512)):
                njc = q0 // 128 + 4
                o_ps = [
                    psumo.tile([128, 512], f32, tag="ps" if psumo is psum else "ops", name=f"o{b}_{h}_{q0}")
                    for h in range(HPC)
                ]
                pvq = []

                def fire(keep, pvq=pvq):
                    while len(pvq) > keep:
                        pvq.pop(0)()

                for jc in range(njc):
                    rel = jc * 128 - q0
                    c0 = max(rel, 0)
                    w = 512 - c0
                    s_ps = psum2.tile([128, 1024], f32, tag="ps" if psum2 is psum else "ps2", name=f"s{b}_{q0}_{jc}")
                    pt = pt_pool.tile([128, 1024], bf16, tag="pt", name=f"p{b}_{q0}_{jc}")
                    for h in range(HPC):
                        nc.tensor.matmul(
                            s_ps[:, h * 512 : h * 512 + w],
                            lhsT=KT[b][h * D : (h + 1) * D, jc * 128 : (jc + 1) * 128],
                            rhs=QT[b][h * D : (h + 1) * D, q0 + c0 : q0 + 512],
                            start=True,
                            stop=True,
                            tile_position=(h * D, 0),
                        )
                    if jc == 0:
                        while pending_fin:
                            pending_fin.pop(0)()
                    if w == 512:
                        nc.scalar.activation(pt[:], s_ps[:], AF.Exp, scale=SCALE)
                    else:
                        for h in range(HPC):
                            nc.scalar.activation(
                                pt[:, h * 512 : h * 512 + w],
                                s_ps[:, h * 512 : h * 512 + w],
                                AF.Exp,
                                scale=SCALE,
                            )
                    if rel >= 0:
                        for h in range(HPC):
                            mul_eng.tensor_mul(
                                pt[:, h * 512 : h * 512 + 128],
                                pt[:, h * 512 : h * 512 + 128],
                                cm_t[:],
                            )
                    fill(njc)
                    fire(CONFIG["lag"])

                    for h in range(HPC):
                        def pv(h=h, q0=q0, jc=jc, njc=njc, o_ps=o_ps[h], pt=pt, c0=c0, w=w, b=b):
                            nc.tensor.matmul(
                                o_ps[0:VW, c0:512],
                                lhsT=vaug[b][:, (h * JC + jc) * VW : (h * JC + jc + 1) * VW],
                                rhs=pt[:, h * 512 : h * 512 + w],
                                start=(jc == 0),
                                stop=(jc == njc - 1),
                            )

                        pvq.append(pv)
                fire(0)
                fill()

                for h in range(HPC):
                    hp = slice(h * D, (h + 1) * D)
                    ops = o_ps[h]
                    # 1/denom = exp(-ln(denom)) on ACT: ln and exp share the
                    # natural_log_exp_and_others table (no table-swap cost);
                    # DVE reciprocal on a [1,512] AP would be 3.3us.
                    t1 = r_pool.tile([1, 512], f32, tag="t1", name=f"t1{b}_{h}_{q0}")
                    nc.scalar.activation(t1[:], ops[D : D + 1, :], AF.Ln)
                    r1 = r_pool.tile([1, 512], bf16, tag="r1", name=f"r1{b}_{h}_{q0}")
                    nc.scalar.activation(r1[:], t1[:], AF.Exp, scale=-1.0)
                    ot_raw = pt_pool.tile([D, 512], f32, tag="ot_raw", name=f"or{b}_{h}_{q0}")
                    nc.vector.tensor_copy(ot_raw[:], ops[0:D, :])

                    def fin(b=b, hp=hp, q0=q0, o_ps=ops, r1=r1, ot_raw=ot_raw):
                        # rank-1 PE broadcast of 1/denom into the spare rows
                        # of the same PSUM bank, then the divide. Deferred to
                        # the next block's start so the PE queue never
                        # head-of-line blocks on the reciprocal.
                        nc.tensor.matmul(
                            o_ps[D : D + D, :],
                            lhsT=ones_r[:],
                            rhs=r1[:],
                            start=True,
                            stop=True,
                        )
                        nc.vector.tensor_mul(
                            OT[b][hp, q0 : q0 + 512], ot_raw[:], o_ps[D : D + D, :]
                        )

                    pending_fin.append(fin)
                if post_block is not None:
                    filler.extend(post_block(q0))
            # drain leftovers: fins FIRST — leftover fillers include outproj
            # ops that read the OT slices the deferred fins write
            while pending_fin:
                pending_fin.pop(0)()
            while filler:
                filler.pop(0)()

        NCHUNKS = sum(q0 // 128 + 4 for q0 in range(0, N, 512))  # 40 per b

        if not CONFIG["inter"]:
            for b in range(B):
                for u in qkv_microops(b):
                    u()
                emit_attention(b, [], rate=0.0)
                for u in outproj_microops(b):
                    u()
        else:
            for u in qkv_microops(0):
                u()
            f2 = qkv_microops(1)
            tail1 = []
            if CONFIG["defer_nb3"]:
                tail1 = [u for u in f2 if getattr(u, "_nb", 0) == 3]
                f2 = [u for u in f2 if getattr(u, "_nb", 0) != 3]
            emit_attention(0, f2, rate=len(f2) / NCHUNKS)
            f3 = tail1 + outproj_microops(0)
            op1 = outproj_microops(1)
            if CONFIG["op1fill"]:
                def release_op1(q0):
                    lo = q0 // 512 * 8
                    return op1[lo : lo + 8]
                emit_attention(
                    1, f3, rate=(len(f3) + len(op1)) / NCHUNKS, post_block=release_op1
                )
            else:
                emit_attention(1, f3, rate=len(f3) / NCHUNKS)
                for u in op1:
                    u()
    return nc


NP_BF16 = mybir.dt.np(bf16)


def prep_in_maps(x, Wqkv, bqkv, Wout, bout):
    x = np.ascontiguousarray(np.asarray(x, dtype=np.float32))
    Wqkv = np.asarray(Wqkv, dtype=np.float32)
    bqkv = np.asarray(bqkv, dtype=np.float32)
    Wout = np.asarray(Wout, dtype=np.float32)

    # xb: block (b, nb) as a contiguous [128, EC*512] slab; xb[slab+p, e*512+t]
    # = x[b, nb*512+t, e*128+p]
    xb = np.ascontiguousarray(
        x.reshape(B, NB, 512, EC, 128).transpose(0, 1, 4, 3, 2).reshape(B * NB * 128, EC * 512)
    ).astype(NP_BF16)
    Wr = Wqkv.reshape(H, D, 3, E)
    br = bqkv.reshape(H, D, 3)

    j = np.arange(128)[:, None]
    q = np.arange(128)[None, :]
    cmask = (j <= q).astype(NP_BF16)

    in_maps = []
    for c in range(NCORES):
        hs = [HPC * c + i for i in range(HPC)]
        wqkv_c = np.concatenate(
            [
                np.concatenate([Wr[h, :, p, :].T for h in hs], axis=1)
                for p in range(3)
            ],
            axis=1,
        )  # [E, 384]
        bias2_c = np.stack(
            [np.concatenate([br[h, :, p] for h in hs]) for p in range(3)], axis=1
        )  # [128, 3]
        wo_c = np.ascontiguousarray(Wout[:, hs[0] * D : (hs[-1] + 1) * D].T)  # [128, E]
        # wqkv_c is [E, 384] = [(e p), c]; repack to [p, (e c)] to match wt
        wqkv_p = wqkv_c.reshape(EC, 128, 3 * D2).transpose(1, 0, 2).reshape(128, EC * 3 * D2)
        in_maps.append(
            {
                "xb": xb,
                "wqkv": np.ascontiguousarray(wqkv_p).astype(NP_BF16),
                "bias2": np.ascontiguousarray(bias2_c),
                "wo": wo_c.astype(NP_BF16),
                "cmask": cmask,
            }
        )
    return in_maps


def assemble_output(results, bout):
    bout = np.asarray(bout, dtype=np.float32)
    y = np.zeros((B * N, E), dtype=np.float32)
    for res in results:
        y += np.asarray(res["y"], dtype=np.float32)
    y += bout[None, :]
    return y.reshape(B, N, E)


# ---------------------------------------------------------------------------
# NTFF profile hook shim (used only when _TRACE is set by a test harness)
# ---------------------------------------------------------------------------
import contextlib
import ctypes
import sys
import types

_SO_PATH = "/opt/axon/libaxon_pjrt.so"
_hook = None


def _make_hook():
    lib = ctypes.CDLL(_SO_PATH)
    if not hasattr(lib, "axon_start_nrt_profile"):
        return None
    lib.axon_start_nrt_profile.argtypes = [
        ctypes.POINTER(ctypes.c_int64),
        ctypes.c_size_t,
    ]
    lib.axon_start_nrt_profile.restype = ctypes.c_int64
    lib.axon_stop_nrt_profile.argtypes = [ctypes.c_char_p]
    lib.axon_stop_nrt_profile.restype = ctypes.c_int64

    @contextlib.contextmanager
    def _profile(output_dir, device_ids):
        import jax

        jax.devices()
        if device_ids:
            ids = (ctypes.c_int64 * len(device_ids))(*device_ids)
            rc = lib.axon_start_nrt_profile(ids, len(device_ids))
        else:
            rc = lib.axon_start_nrt_profile(None, 0)
        if rc != 0:
            raise RuntimeError(f"axon_start_nrt_profile rc={rc}")
        try:
            yield
        finally:
            n = lib.axon_stop_nrt_profile(str(output_dir).encode())
            if n < 0:
                raise RuntimeError(f"axon_stop_nrt_profile rc={n}")
            print(f"profile: {n} file(s) written to {output_dir}")

    return _profile


def _install_ntff_hook():
    global _hook
    _hook = _make_hook()
    mod = types.ModuleType("antenv.axon_hooks")
    mod.get_axon_ntff_profile_hook = lambda: _hook

    def set_axon_ntff_profile_hook(h):
        global _hook
        _hook = h

    mod.set_axon_ntff_profile_hook = set_axon_ntff_profile_hook
    sys.modules["antenv.axon_hooks"] = mod


# ---------------------------------------------------------------------------
# host-side entry point
# ---------------------------------------------------------------------------
_NC = None
_TRACE = False
_TRACE_DIR = "/tmp/attn_kernel_trace"
LAST_EXEC_NS = None


def _get_nc():
    global _NC
    if _NC is None:
        _apply_tile_drain_patch()
        nc = build_nc()
        _split_excess_waits(nc)
        _NC = nc
    return _NC


def kernel(x, Wqkv, bqkv, Wout, bout):
    global LAST_EXEC_NS
    from concourse.bass_utils import run_bass_kernel_spmd

    nc = _get_nc()
    in_maps = prep_in_maps(x, Wqkv, bqkv, Wout, bout)
    kwargs = {}
    if _TRACE:
        import shutil

        shutil.rmtree(_TRACE_DIR, ignore_errors=True)
        _install_ntff_hook()
        kwargs = {"trace": True, "tmpdir": _TRACE_DIR}
    res = run_bass_kernel_spmd(nc, in_maps, list(range(NCORES)), **kwargs)
    LAST_EXEC_NS = res.exec_time_ns
    return assemble_output(res.results, bout)
